# revision 6
# baseline (speedup 1.0000x reference)
"""Batch Child-Sum TreeLSTM cell on 8 Trainium2 NeuronCores.

Strategy (data-parallel over nodes):
  - Shard the N nodes (and their contiguous child segments) evenly across the
    8 cores; replicate the small weight matrices. Irregular sorted segment_ids
    are regularized host-side by zero-padding every node to max_children
    slots (exact: padded slots contribute 0).
  - Feature-major staging (features on SBUF partitions, nodes on the free
    axis). x is staged e3m4 (halves its HBM traffic; adds ~1e-2 h-rel-err vs
    the 2e-2 gate), child_h / child_c / outputs fp16, weights fp16
    (mixed-dtype matmuls are legal as long as neither side is fp32).
  - Engine balance per 512-node sub-tile:
      PE   18 matmuls (z-gates 3x[x0,x1,ht]; f-gates 3x[x0,x1,Uf.ch_c])
      ACT  2 merged sigmoids (zio 2-bank PSUM, f3 3-bank PSUM -> G slab)
           + tanh([zu_evac_s | c_{s-1}]) paired in one strided instr
      DVE  ht finish-add, zu evac (PSUM->SBUF), one 4D-AP multiply
           [sig_i|f0|f1|f2] (.) [tanh_u|cc0|cc1|cc2], two tree adds -> c
      GPS  macro-level ht01=ch0+ch1, h = sig_o (.) tanh_c, store triggers
           (h-mul and stores for macro m are emitted at macro m+1's top so
           ht01_{m+1} is never queued behind them in the gpsimd FIFO)
  - Layouts that make the fused APs single instructions:
      G slab per sub (pitch 2560): [sig_o|sig_i|f0|f1|f2], 512 pitch
      M slab per macro: [tanh_c(msz) | tanh_u(msz) | cc0 | cc1 | cc2]
        -> DVE big-mul B side = 4 slabs at uniform stride msz
        -> paired tanh dst [tanh_c_{s-1}@s0-ssz | tanh_u_s@msz+s0],
           src [c_{s-1} | zu_evac_s] in CO = [c(msz) | zu_evac(msz) | pad]
           both with uniform stride msz+ssz
"""

from contextlib import ExitStack

import numpy as np

import concourse.bass as bass
import concourse.bacc as bacc
import concourse.tile as tile
from concourse import mybir
from concourse.bass_utils import run_bass_kernel_spmd

F32 = mybir.dt.float32
FP16 = mybir.dt.float16
FP8 = mybir.dt.float8e3  # e3m4: 4-bit mantissa; |x|max ~5.4 well in range

N_CORES = 8

SUB = 512           # nodes per sub-tile == one PSUM bank of fp32
SUBS_PER_MACRO = 4  # macro = DMA/SBUF granularity
MACRO = SUB * SUBS_PER_MACRO

# Engine-assignment knobs (fallbacks if gpsimd fp16 tensor ops misbehave)
HT01_ON_GPSIMD = True
HMUL_ON_GPSIMD = True
PAIR_TANH = True
X_FP8 = True


def _chunks(total, step):
    out = []
    off = 0
    while off < total:
        out.append((off, min(step, total - off)))
        off += step
    return out


def build_program(npc, in_dim, hid, cpn, zero_bias=True):
    """Bass program for one core's shard: npc nodes, npc*cpn edges."""
    assert in_dim == 256 and hid == 128 and cpn == 3
    kx = in_dim // 128  # k-chunks of x
    XDT = FP8 if X_FP8 else FP16

    nc = bacc.Bacc("TRN2", target_bir_lowering=False, debug=False)

    x8d = nc.dram_tensor("x8", [128, kx * npc], XDT, kind="ExternalInput").ap()
    chd = nc.dram_tensor("chT", [hid, cpn * npc], FP16, kind="ExternalInput").ap()
    # ccS: host-interleaved per (macro, sub): [cc0_s | cc1_s | cc2_s]
    ccd = nc.dram_tensor("ccS", [hid, cpn * npc], FP16, kind="ExternalInput").ap()
    wcxd = nc.dram_tensor("Wcx", [128, 3 * kx * 128], FP16, kind="ExternalInput").ap()
    wchtd = nc.dram_tensor("Wcht", [128, 3 * 128], FP16, kind="ExternalInput").ap()
    wfxd = nc.dram_tensor("Wfx", [128, kx * 128], FP16, kind="ExternalInput").ap()
    ufd = nc.dram_tensor("Uf", [128, 128], FP16, kind="ExternalInput").ap()
    bcd = nc.dram_tensor("bc", [hid, 3], F32, kind="ExternalInput").ap()
    bfd = nc.dram_tensor("bf", [hid, 1], F32, kind="ExternalInput").ap()

    cTd = nc.dram_tensor("cT", [hid, npc], FP16, kind="ExternalOutput").ap()
    hTd = nc.dram_tensor("hT", [hid, npc], FP16, kind="ExternalOutput").ap()

    x8d3 = x8d.rearrange("p (k n) -> p k n", k=kx)
    chd3 = chd.rearrange("p (c n) -> p c n", c=cpn)

    ACTF = mybir.ActivationFunctionType
    SIG = ACTF.Sigmoid
    TANH = ACTF.Tanh

    with tile.TileContext(nc) as tc, ExitStack() as ctx:
        consts = ctx.enter_context(tc.tile_pool(name="consts", bufs=1))
        dma_pool = ctx.enter_context(tc.tile_pool(name="dmain", bufs=3))
        mac_pool = ctx.enter_context(tc.tile_pool(name="macro", bufs=2))
        work = ctx.enter_context(tc.tile_pool(name="work", bufs=3))
        psum = ctx.enter_context(tc.tile_pool(name="psum", bufs=1, space="PSUM"))

        # ---- resident weights ----
        wcx = consts.tile([128, 3 * kx * 128], FP16, tag="wcx")
        nc.sync.dma_start(out=wcx, in_=wcxd)
        wcx4 = wcx.rearrange("p (g k m) -> p g k m", g=3, k=kx)
        wcht = consts.tile([128, 3 * 128], FP16, tag="wcht")
        nc.sync.dma_start(out=wcht, in_=wchtd)
        wcht3 = wcht.rearrange("p (g m) -> p g m", g=3)
        wfx = consts.tile([128, kx * 128], FP16, tag="wfx")
        nc.sync.dma_start(out=wfx, in_=wfxd)
        wfx3 = wfx.rearrange("p (k m) -> p k m", k=kx)
        uf = consts.tile([128, 128], FP16, tag="uf")
        nc.sync.dma_start(out=uf, in_=ufd)
        bc_sb = consts.tile([128, 3], F32, tag="bc")
        nc.sync.dma_start(out=bc_sb, in_=bcd)
        bf_sb = consts.tile([128, 1], F32, tag="bf")
        nc.sync.dma_start(out=bf_sb, in_=bfd)

        # ---- PSUM (8 banks): zio x2 (4) + f3 (3) + zu (1) ----
        zio_a = psum.tile([128, 1024], F32, tag="zio0")
        zio_b = psum.tile([128, 1024], F32, tag="zio1")
        zio_t = [zio_a, zio_b]
        f3_t = psum.tile([128, 1536], F32, tag="f3")
        zu_t = psum.tile([128, 512], F32, tag="zu")

        state = {"prev": None}

        def flush_prev():
            if state["prev"] is None:
                return
            pm0, pmsz, psubs, G, M2q, CO, HO = state["prev"]
            heng = nc.gpsimd if HMUL_ON_GPSIMD else nc.vector
            # h = sig_o (.) tanh_c ; subs grouped by uniform ssz
            groups = {}
            for s, (s0, ssz) in enumerate(psubs):
                groups.setdefault(ssz, []).append(s)
            for ssz, idxs in groups.items():
                s_lo, s_hi = idxs[0], idxs[-1]
                ns = s_hi - s_lo + 1
                so_ap = (
                    G[:, 2560 * s_lo : 2560 * (s_hi + 1)]
                    .rearrange("p (s j) -> p s j", s=ns)[:, :, 0:ssz]
                )
                tc_ap = M2q[:, s_lo : s_hi + 1, 4, 0:ssz]
                ho_ap = (
                    HO[:, psubs[s_lo][0] : psubs[s_lo][0] + ns * ssz]
                    .rearrange("p (s j) -> p s j", s=ns)
                )
                heng.tensor_mul(ho_ap, so_ap, tc_ap)
            nc.gpsimd.dma_start(out=cTd[:, pm0 : pm0 + pmsz], in_=CO)
            nc.gpsimd.dma_start(out=hTd[:, pm0 : pm0 + pmsz], in_=HO)
            state["prev"] = None

        for m0, msz in _chunks(npc, MACRO):
            subs = _chunks(msz, SUB)
            nsub = len(subs)

            # ---- input DMAs (SP HWDGE queue) ----
            x8_m = dma_pool.tile([128, kx * msz], XDT, tag="x8")
            x8_m3 = x8_m.rearrange("p (k n) -> p k n", k=kx)
            for k in range(kx):
                nc.sync.dma_start(out=x8_m3[:, k, :], in_=x8d3[:, k, m0 : m0 + msz])
            ch_m = dma_pool.tile([128, cpn * msz], FP16, tag="ch")
            ch_m3 = ch_m.rearrange("p (c n) -> p c n", c=cpn)
            for c in range(cpn):
                nc.sync.dma_start(out=ch_m3[:, c, :], in_=chd3[:, c, m0 : m0 + msz])

            # M2: per-sub 2560 slabs [tanh_u | cc0 | cc1 | cc2 | tanh_c]
            M2 = mac_pool.tile([128, 2560 * nsub], FP16, tag="M2")
            M2q = M2.rearrange("p (s q j) -> p s q j", q=5, j=512)
            # one cc DMA per uniform-ssz group of subs
            groups = {}
            for s, (s0, ssz) in enumerate(subs):
                groups.setdefault(ssz, []).append(s)
            for ssz, idxs in groups.items():
                s_lo, s_hi = idxs[0], idxs[-1]
                ns = s_hi - s_lo + 1
                src = (
                    ccd[:, cpn * (m0 + subs[s_lo][0]) : cpn * (m0 + subs[s_lo][0]) + ns * cpn * ssz]
                    .rearrange("p (s c j) -> p s c j", s=ns, c=cpn)
                )
                nc.sync.dma_start(out=M2q[:, s_lo : s_hi + 1, 1:4, 0:ssz], in_=src)

            G = mac_pool.tile([128, 2560 * nsub], FP16, tag="G")
            CO = mac_pool.tile([128, msz], FP16, tag="CO")
            HO = mac_pool.tile([128, msz], FP16, tag="HO")
            ht01 = mac_pool.tile([128, msz], FP16, tag="ht01")
            ht_m = mac_pool.tile([128, msz], FP16, tag="ht_m")

            # gpsimd: ht01 for THIS macro first, then deferred prev-macro work
            ht01_eng = nc.gpsimd if HT01_ON_GPSIMD else nc.vector
            ht01_eng.tensor_add(ht01, ch_m3[:, 0, :], ch_m3[:, 1, :])
            flush_prev()
            # DVE: finish ht for the whole macro
            nc.vector.tensor_add(ht_m, ht01, ch_m3[:, 2, :])

            for si, (s0, ssz) in enumerate(subs):
                g0 = 2560 * si
                zio = zio_t[si % 2]

                # ---- PE: zio groups, then f, then zu ----
                for g in (0, 1):
                    dst = zio[:, 0:ssz] if g == 0 else zio[:, 512 : 512 + ssz]
                    for k in range(kx):
                        nc.tensor.matmul(
                            dst,
                            lhsT=wcx4[:, g, k, :],
                            rhs=x8_m3[:, k, s0 : s0 + ssz],
                            start=(k == 0),
                            stop=False,
                        )
                    nc.tensor.matmul(
                        dst, lhsT=wcht3[:, g, :], rhs=ht_m[:, s0 : s0 + ssz],
                        start=False, stop=True,
                    )
                for c in range(cpn):
                    dst = f3_t[:, 512 * c : 512 * c + ssz]
                    for k in range(kx):
                        nc.tensor.matmul(
                            dst,
                            lhsT=wfx3[:, k, :],
                            rhs=x8_m3[:, k, s0 : s0 + ssz],
                            start=(k == 0),
                            stop=False,
                        )
                    nc.tensor.matmul(
                        dst, lhsT=uf, rhs=ch_m3[:, c, s0 : s0 + ssz],
                        start=False, stop=True,
                    )
                for k in range(kx):
                    nc.tensor.matmul(
                        zu_t[:, 0:ssz],
                        lhsT=wcx4[:, 2, k, :],
                        rhs=x8_m3[:, k, s0 : s0 + ssz],
                        start=(k == 0),
                        stop=False,
                    )
                nc.tensor.matmul(
                    zu_t[:, 0:ssz], lhsT=wcht3[:, 2, :], rhs=ht_m[:, s0 : s0 + ssz],
                    start=False, stop=True,
                )

                # ---- ACT: sigf first (frees f3 early), tanh_u, sigzio ----
                fbias = {} if zero_bias else {"bias": bf_sb[:, 0:1]}
                nc.scalar.activation(
                    G[:, g0 + 1024 : g0 + 2048 + ssz], f3_t[:, 0 : 1024 + ssz],
                    SIG, **fbias,
                )
                ubias = {} if zero_bias else {"bias": bc_sb[:, 2:3]}
                nc.scalar.activation(
                    M2q[:, si, 0, 0:ssz], zu_t[:, 0:ssz], TANH, **ubias,
                )
                if zero_bias:
                    nc.scalar.activation(
                        G[:, g0 : g0 + 512 + ssz], zio[:, 0 : 512 + ssz], SIG
                    )
                else:
                    nc.scalar.activation(
                        G[:, g0 : g0 + ssz], zio[:, 0:ssz], SIG, bias=bc_sb[:, 0:1]
                    )
                    nc.scalar.activation(
                        G[:, g0 + 512 : g0 + 512 + ssz], zio[:, 512 : 512 + ssz],
                        SIG, bias=bc_sb[:, 1:2],
                    )

                # ---- DVE: big multiply + tree sum -> c ----
                W = work.tile([128, 2048], FP16, tag="W")
                nc.vector.tensor_mul(
                    W, G[:, g0 + 512 : g0 + 2560], M2[:, g0 : g0 + 2048]
                )
                W2 = work.tile([128, 1024], FP16, tag="W2")
                nc.vector.tensor_add(W2, W[:, 0:1024], W[:, 1024:2048])
                nc.vector.tensor_add(
                    CO[:, s0 : s0 + ssz], W2[:, 0:ssz], W2[:, 512 : 512 + ssz]
                )

            # ---- one tanh over the macro's c -> tanh_c slabs ----
            for ssz, idxs in groups.items():
                s_lo, s_hi = idxs[0], idxs[-1]
                ns = s_hi - s_lo + 1
                src = (
                    CO[:, subs[s_lo][0] : subs[s_lo][0] + ns * ssz]
                    .rearrange("p (s j) -> p s j", s=ns)
                )
                nc.scalar.activation(M2q[:, s_lo : s_hi + 1, 4, 0:ssz], src, TANH)

            state["prev"] = (m0, msz, subs, G, M2q, CO, HO)

        flush_prev()

    nc.compile()
    return nc


TRACE = False  # set True (e.g. from test.py) to capture an NTFF profile
LAST_RESULTS = None  # BassKernelResults of the most recent kernel() call

_PROGRAM_CACHE = {}


def _get_program(npc, in_dim, hid, cpn, zero_bias):
    key = (npc, in_dim, hid, cpn, zero_bias,
           SUB, MACRO, HT01_ON_GPSIMD, HMUL_ON_GPSIMD, PAIR_TANH, X_FP8)
    if key not in _PROGRAM_CACHE:
        _PROGRAM_CACHE[key] = build_program(npc, in_dim, hid, cpn, zero_bias)
    return _PROGRAM_CACHE[key]


def _pad_children(child_c, child_h, segment_ids, n):
    """Regularize to exactly max_c children per node (zero padding is exact:
    padded slots contribute sigmoid(..)*0 to fc and 0 to the child sum)."""
    seg = np.asarray(segment_ids).astype(np.int64)
    e = seg.shape[0]
    counts = np.bincount(seg, minlength=n)
    max_c = int(counts.max()) if e else 1
    if e == n * max_c and np.all(counts == max_c):
        return child_c, child_h, max_c  # already regular (and sorted)
    hid = child_h.shape[1]
    slot = np.arange(e, dtype=np.int64) - np.repeat(
        np.concatenate([[0], np.cumsum(counts)[:-1]]), counts
    )
    cc = np.zeros((n * max_c, hid), np.float32)
    ch = np.zeros((n * max_c, hid), np.float32)
    idx = seg * max_c + slot
    cc[idx] = child_c
    ch[idx] = child_h
    return cc, ch, max_c


def kernel(
    inputs,
    child_c,
    child_h,
    segment_ids,
    W_combined,
    b_combined,
    W_f,
    U_f,
    b_f,
):
    import ml_dtypes

    inputs = np.asarray(inputs, dtype=np.float32)
    child_c = np.asarray(child_c, dtype=np.float32)
    child_h = np.asarray(child_h, dtype=np.float32)
    W_combined = np.asarray(W_combined, dtype=np.float32)
    b_combined = np.asarray(b_combined, dtype=np.float32)
    W_f = np.asarray(W_f, dtype=np.float32)
    U_f = np.asarray(U_f, dtype=np.float32)
    b_f = np.asarray(b_f, dtype=np.float32)

    n, in_dim = inputs.shape
    hid = U_f.shape[0]
    kx = in_dim // 128

    child_c, child_h, cpn = _pad_children(child_c, child_h, segment_ids, n)

    assert n % N_CORES == 0
    npc = n // N_CORES

    zero_bias = bool(np.all(b_combined == 0) and np.all(b_f == 0))
    nc = _get_program(npc, in_dim, hid, cpn, zero_bias)

    xdt = ml_dtypes.float8_e3m4 if X_FP8 else np.float16

    # ---- weights (gate order o, i, u in columns of W_combined: i|o|u) ----
    gates = [slice(128, 256), slice(0, 128), slice(256, 384)]  # o, i, u
    Wcx = np.stack(
        [
            np.stack([W_combined[128 * k : 128 * (k + 1), cg] for k in range(kx)], 0)
            for cg in gates
        ],
        0,
    )  # [3, kx, 128(p... rows), 128(m)] where rows are the k-chunk rows
    Wcx = np.ascontiguousarray(Wcx.transpose(2, 0, 1, 3).reshape(128, -1)).astype(
        np.float16
    )
    Wcht = np.stack([W_combined[256:384, cg] for cg in gates], 0)  # [3, 128p, 128m]
    Wcht = np.ascontiguousarray(Wcht.transpose(1, 0, 2).reshape(128, -1)).astype(
        np.float16
    )
    Wfx = np.stack([W_f[128 * k : 128 * (k + 1), :] for k in range(kx)], 0)
    Wfx = np.ascontiguousarray(Wfx.transpose(1, 0, 2).reshape(128, -1)).astype(
        np.float16
    )
    Uf = np.ascontiguousarray(U_f).astype(np.float16)
    b3 = b_combined.reshape(3, hid)  # rows i, o, u
    bc = np.ascontiguousarray(np.stack([b3[1], b3[0], b3[2]], 1)).astype(np.float32)
    bf = np.ascontiguousarray(b_f.reshape(hid, 1)).astype(np.float32)

    # ccS interleave order: per (macro, sub): [cc0_sub | cc1_sub | cc2_sub]
    # Build a column permutation of the child-major [cpn, npc] layout.
    perm_c = []
    perm_n = []
    for m0, msz in _chunks(npc, MACRO):
        for s0, ssz in _chunks(msz, SUB):
            for ch_i in range(cpn):
                perm_c.append(np.full(ssz, ch_i, np.int64))
                perm_n.append(np.arange(m0 + s0, m0 + s0 + ssz, dtype=np.int64))
    perm_c = np.concatenate(perm_c)
    perm_n = np.concatenate(perm_n)

    in_maps = []
    for c in range(N_CORES):
        n0, n1 = c * npc, (c + 1) * npc
        e0, e1 = n0 * cpn, n1 * cpn
        xs = inputs[n0:n1].T  # [in_dim, npc]
        x8 = np.ascontiguousarray(
            xs.reshape(kx, 128, npc).transpose(1, 0, 2).reshape(128, kx * npc)
        ).astype(xdt)
        cc3 = child_c[e0:e1].reshape(npc, cpn, hid).transpose(2, 1, 0)  # [hid,cpn,npc]
        ccS = np.ascontiguousarray(cc3[:, perm_c, perm_n]).astype(np.float16)
        in_maps.append(
            {
                "x8": x8,
                "chT": np.ascontiguousarray(
                    child_h[e0:e1]
                    .reshape(npc, cpn, hid)
                    .transpose(2, 1, 0)
                    .astype(np.float16)
                ).reshape(hid, npc * cpn),
                "ccS": ccS,
                "Wcx": Wcx,
                "Wcht": Wcht,
                "Wfx": Wfx,
                "Uf": Uf,
                "bc": bc,
                "bf": bf,
            }
        )

    res = run_bass_kernel_spmd(
        nc, in_maps, core_ids=list(range(N_CORES)), trace=TRACE
    )
    global LAST_RESULTS
    LAST_RESULTS = res

    c_full = np.empty((n, hid), np.float32)
    h_full = np.empty((n, hid), np.float32)
    for c in range(N_CORES):
        n0, n1 = c * npc, (c + 1) * npc
        c_full[n0:n1] = res.results[c]["cT"].T.astype(np.float32)
        h_full[n0:n1] = res.results[c]["hT"].T.astype(np.float32)
    return (c_full, h_full)


if __name__ == "__main__":
    # small smoke test against a numpy reference
    rng = np.random.default_rng(0)
    n, in_dim, hid, cpn = N_CORES * (2 * MACRO + SUB + 424), 256, 128, 3
    e = n * cpn
    inputs = {
        "inputs": rng.standard_normal((n, in_dim), np.float32),
        "child_c": rng.standard_normal((e, hid), np.float32),
        "child_h": rng.standard_normal((e, hid), np.float32),
        "segment_ids": np.repeat(np.arange(n, dtype=np.int64), cpn),
        "W_combined": (rng.standard_normal((in_dim + hid, 3 * hid), np.float32) * 0.02),
        "b_combined": np.zeros(3 * hid, np.float32),
        "W_f": rng.standard_normal((in_dim, hid), np.float32) * 0.02,
        "U_f": rng.standard_normal((hid, hid), np.float32) * 0.02,
        "b_f": np.zeros(hid, np.float32),
    }
    act_c, act_h = kernel(**inputs)

    x = inputs["inputs"]; ch = inputs["child_h"]; cc = inputs["child_c"]
    Wc = inputs["W_combined"]; Wf = inputs["W_f"]; Uf = inputs["U_f"]
    ht = ch.reshape(n, cpn, hid).sum(1)
    z = np.concatenate([x, ht], 1) @ Wc
    zi, zo, zu = z[:, :128], z[:, 128:256], z[:, 256:]
    sig = lambda v: 1 / (1 + np.exp(-v))
    f = sig((x @ Wf)[np.repeat(np.arange(n), cpn)] + ch @ Uf)
    fc = (f * cc).reshape(n, cpn, hid).sum(1)
    c_ref = sig(zi) * np.tanh(zu) + fc
    h_ref = sig(zo) * np.tanh(c_ref)
    for nm, a, r in [("c", act_c, c_ref), ("h", act_h, h_ref)]:
        rel = np.abs(a - r).max() / np.abs(r).max()
        print(f"{nm}: rel={rel:.4e}")
